# revision 8
# baseline (speedup 1.0000x reference)
"""Correntropy loss on 8 Trainium2 NeuronCores.

Reference math (all f32):
    t = (target - 0.5) * 2 ; o = (output - 0.5) * 2
    cost = mean(1 - exp(-sigma * (o - t)^2)),  sigma = 1/1000

Since o - t == 2*(output - target), this equals
    mean(1 - exp(-c * w)),  w = (output - target)^2,  c = 4*sigma = 0.004

Direct evaluation of sum(1 - exp(-c*w)) on device loses ~3 decimal
digits: the f32 running sums of exp(...) ~= 1 carry a systematic
~2e-7 relative rounding bias that the final N - S cancellation
amplifies ~1500x (c*w <= 0.016, so 1-exp is ~6.6e-4 of each summand).

Instead the device computes exact power sums (moments) of w
    S1 = sum(w), S2 = sum(w^2), S3 = sum(w^3)
and the host evaluates the Taylor series in f64:
    sum(1 - exp(-c*w)) = c*S1 - c^2/2*S2 + c^3/6*S3 - O(c^4*S4)
The dropped S4 term is ~9e-8 relative; every device op involved
(ACT Square LUT, DVE multiply) was verified bit-exact on HW, and the
fused f32 accumulators contribute <~2e-7 (S2/S3 enter scaled by
3e-3 / 9e-6 so their accumulation error is irrelevant).

Sharding (per the data-parallel hint): both tensors row-sharded into
8 x [8192, 1000]; each core's two shards are host-interleaved into one
array [n_tiles, 2, 128, 2000] so one DMA per tile fetches both
operands (fewer cross-engine waits). Per core, 32 tiles of [128x4000]:
    DVE: d  = out_half - tgt_half    (tensor_sub)
    ACT: w  = Square(d),  accum -> S1 column   (bit-exact square)
    ACT: w2 = Square(w),  accum -> S2 column
    DVE: w3 = (w*1)*w2,   accum -> S3 column   (scalar_tensor_tensor)
Partial sums land in a [128, 96] tile, DMA'd out; host reduces in f64
and applies the series. The scalar "all-reduce" of the hint happens on
the host (8 tiny [128,96] arrays), which is exact.
"""

import numpy as np

import concourse.bacc as bacc
import concourse.mybir as mybir
import concourse.tile as tile
from concourse.bass_utils import run_bass_kernel_spmd

N_CORES = 8
ROWS = 65536
COLS = 1000
ROWS_PER_CORE = ROWS // N_CORES  # 8192
P = 128  # SBUF partitions

Q = 2  # rows folded into the free dim per partition
FREE = Q * COLS  # 2000 elements of one operand per partition per tile
N_TILES = ROWS_PER_CORE // (P * Q)  # 32
ACC_COLS = 3 * N_TILES  # S1 | S2 | S3 column blocks

F32 = mybir.dt.float32


def _build():
    nc = bacc.Bacc()
    comb_p = nc.declare_dram_parameter(
        "combined", [N_TILES * 2 * P, FREE], F32, isOutput=False
    )
    acc_p = nc.declare_dram_parameter("partial", [P, ACC_COLS], F32, isOutput=True)

    # [n_tiles, 2, P, FREE] -> per-tile [P, 2, FREE] access pattern
    comb_v = comb_p[:].rearrange("(t c p) m -> t p c m", c=2, p=P)

    with tile.TileContext(nc) as tc:
        with (
            tc.tile_pool(name="io", bufs=6) as io_pool,
            tc.tile_pool(name="work", bufs=1) as work_pool,
            tc.tile_pool(name="accp", bufs=1) as acc_pool,
        ):
            acc = acc_pool.tile([P, ACC_COLS], F32)
            for t in range(N_TILES):
                ab = io_pool.tile([P, 2 * FREE], F32, tag="ab")
                nc.sync.dma_start(
                    out=ab[:].rearrange("p (c m) -> p c m", c=2), in_=comb_v[t]
                )
                d = work_pool.tile([P, FREE], F32, tag="d", bufs=2)
                nc.vector.tensor_sub(d[:], ab[:, :FREE], ab[:, FREE:])
                w = work_pool.tile([P, FREE], F32, tag="w", bufs=3)
                nc.scalar.activation(
                    w[:],
                    d[:],
                    mybir.ActivationFunctionType.Square,
                    accum_out=acc[:, t : t + 1],
                )
                w2 = work_pool.tile([P, FREE], F32, tag="w2", bufs=3)
                nc.scalar.activation(
                    w2[:],
                    w[:],
                    mybir.ActivationFunctionType.Square,
                    accum_out=acc[:, N_TILES + t : N_TILES + t + 1],
                )
                w3 = work_pool.tile([P, FREE], F32, tag="w3", bufs=2)
                nc.vector.scalar_tensor_tensor(
                    out=w3[:],
                    in0=w[:],
                    scalar=1.0,
                    in1=w2[:],
                    op0=mybir.AluOpType.mult,
                    op1=mybir.AluOpType.mult,
                    accum_out=acc[:, 2 * N_TILES + t : 2 * N_TILES + t + 1],
                )
            nc.sync.dma_start(out=acc_p[:], in_=acc[:])
    nc.finalize()
    return nc


_NC = None


def _get_nc():
    global _NC
    if _NC is None:
        _NC = _build()
    return _NC


def _shard_inputs(output, target):
    output = np.asarray(output, dtype=np.float32)
    target = np.asarray(target, dtype=np.float32)
    in_maps = []
    for i in range(N_CORES):
        sl = slice(i * ROWS_PER_CORE, (i + 1) * ROWS_PER_CORE)
        o4 = output[sl].reshape(N_TILES, P, FREE)
        t4 = target[sl].reshape(N_TILES, P, FREE)
        comb = np.stack([o4, t4], axis=1).reshape(N_TILES * 2 * P, FREE)
        in_maps.append({"combined": comb})
    return in_maps


def run_device(output, target, trace=False):
    """Returns (per-core partial moment arrays, BassKernelResults)."""
    in_maps = _shard_inputs(output, target)
    res = run_bass_kernel_spmd(_get_nc(), in_maps, list(range(N_CORES)), trace=trace)
    partials = [res.results[i]["partial"] for i in range(N_CORES)]
    return partials, res


def _reduce(partials):
    s1 = s2 = s3 = 0.0
    for p in partials:
        p64 = p.astype(np.float64)
        s1 += p64[:, 0:N_TILES].sum()
        s2 += p64[:, N_TILES : 2 * N_TILES].sum()
        s3 += p64[:, 2 * N_TILES :].sum()
    c = 4.0 * float(np.float32(1.0 / COLS))  # match reference's f32 sigma
    total = c * s1 - (c * c / 2.0) * s2 + (c * c * c / 6.0) * s3
    n = float(ROWS) * float(COLS)
    return np.array(total / n, dtype=np.float32)


def kernel(output, target):
    partials, _ = run_device(output, target)
    return _reduce(partials)


# revision 9
# speedup vs baseline: 1.1891x; 1.1891x over previous
"""Correntropy loss on 8 Trainium2 NeuronCores.

Reference math (all f32):
    t = (target - 0.5) * 2 ; o = (output - 0.5) * 2
    cost = mean(1 - exp(-sigma * (o - t)^2)),  sigma = 1/1000

Since o - t == 2*(output - target), this equals
    mean(1 - exp(-c * w)),  w = (output - target)^2,  c = 4*sigma = 0.004

Direct evaluation of sum(1 - exp(-c*w)) on device loses ~3 decimal
digits: the f32 running sums of exp(...) ~= 1 carry a systematic
~2e-7 relative rounding bias that the final N - S cancellation
amplifies ~1500x (c*w <= 0.016, so 1-exp is ~6.6e-4 of each summand).

Instead the device computes exact power sums (moments) of w
    S1 = sum(w), S2 = sum(w^2), S3 = sum(w^3)
and the host evaluates the Taylor series in f64:
    sum(1 - exp(-c*w)) = c*S1 - c^2/2*S2 + c^3/6*S3 - O(c^4*S4)
The dropped S4 term is ~9e-8 relative; every device op involved
(ACT Square LUT, DVE multiply) was verified bit-exact on HW, and the
fused f32 accumulators contribute <~2e-7 (S2/S3 enter scaled by
3e-3 / 9e-6 so their accumulation error is irrelevant).

Sharding (per the data-parallel hint): both tensors row-sharded into
8 x [8192, 1000]; each core's two shards are host-interleaved into one
array [n_tiles, 2, 128, 2000] so one DMA per tile fetches both
operands (fewer cross-engine waits). Per core, 32 tiles of [128x4000]:
    DVE: d  = out_half - tgt_half    (tensor_sub)
    ACT: w  = Square(d),  accum -> S1 column   (bit-exact square)
    ACT: w2 = Square(w),  accum -> S2 column
    DVE: w3 = (w*1)*w2,   accum -> S3 column   (scalar_tensor_tensor)
Partial sums land in a [128, 96] tile, DMA'd out; host reduces in f64
and applies the series. The scalar "all-reduce" of the hint happens on
the host (8 tiny [128,96] arrays), which is exact.
"""

import numpy as np

import concourse.bacc as bacc
import concourse.mybir as mybir
import concourse.tile as tile
from concourse.bass_utils import run_bass_kernel_spmd

N_CORES = 8
ROWS = 65536
COLS = 1000
ROWS_PER_CORE = ROWS // N_CORES  # 8192
P = 128  # SBUF partitions

Q = 2  # rows folded into the free dim per partition
FREE = Q * COLS  # 2000 elements of one operand per partition per tile
N_TILES = ROWS_PER_CORE // (P * Q)  # 32

# Tail taper: the last DRAM tiles are processed in shrinking slices so the
# serial sub->sq->sq->mul chain after the final DMA is short (~2us vs ~10us).
# (dram_tile, col_offset, width) pieces; widths sum to N_TILES * FREE.
PIECES = [(t, 0, FREE) for t in range(N_TILES - 2)]
PIECES += [(N_TILES - 2, 0, 1000), (N_TILES - 2, 1000, 1000)]
PIECES += [(N_TILES - 1, 0, 500), (N_TILES - 1, 500, 500)]
PIECES += [(N_TILES - 1, 1000 + i * 250, 250) for i in range(4)]
N_PIECES = len(PIECES)  # 38
ACC_COLS = 3 * N_PIECES  # S1 | S2 | S3 column blocks

F32 = mybir.dt.float32


def _build():
    nc = bacc.Bacc()
    comb_p = nc.declare_dram_parameter(
        "combined", [N_TILES * 2 * P, FREE], F32, isOutput=False
    )
    acc_p = nc.declare_dram_parameter("partial", [P, ACC_COLS], F32, isOutput=True)

    # [n_tiles, 2, P, FREE] -> per-tile [P, 2, FREE] access pattern
    comb_v = comb_p[:].rearrange("(t c p) m -> t p c m", c=2, p=P)

    with tile.TileContext(nc) as tc:
        with (
            tc.tile_pool(name="io", bufs=6) as io_pool,
            tc.tile_pool(name="work", bufs=1) as work_pool,
            tc.tile_pool(name="accp", bufs=1) as acc_pool,
        ):
            acc = acc_pool.tile([P, ACC_COLS], F32)
            for i, (t, off, z) in enumerate(PIECES):
                ab = io_pool.tile([P, 2 * z], F32, tag="ab")
                nc.sync.dma_start(
                    out=ab[:].rearrange("p (c m) -> p c m", c=2),
                    in_=comb_v[t][:, :, off : off + z],
                )
                d = work_pool.tile([P, z], F32, tag="d", bufs=2)
                nc.vector.tensor_sub(d[:], ab[:, :z], ab[:, z:])
                w = work_pool.tile([P, z], F32, tag="w", bufs=3)
                nc.scalar.activation(
                    w[:],
                    d[:],
                    mybir.ActivationFunctionType.Square,
                    accum_out=acc[:, i : i + 1],
                )
                w2 = work_pool.tile([P, z], F32, tag="w2", bufs=3)
                nc.scalar.activation(
                    w2[:],
                    w[:],
                    mybir.ActivationFunctionType.Square,
                    accum_out=acc[:, N_PIECES + i : N_PIECES + i + 1],
                )
                w3 = work_pool.tile([P, z], F32, tag="w3", bufs=2)
                nc.vector.scalar_tensor_tensor(
                    out=w3[:],
                    in0=w[:],
                    scalar=1.0,
                    in1=w2[:],
                    op0=mybir.AluOpType.mult,
                    op1=mybir.AluOpType.mult,
                    accum_out=acc[:, 2 * N_PIECES + i : 2 * N_PIECES + i + 1],
                )
            nc.sync.dma_start(out=acc_p[:], in_=acc[:])
    nc.finalize()
    return nc


_NC = None


def _get_nc():
    global _NC
    if _NC is None:
        _NC = _build()
    return _NC


def _shard_inputs(output, target):
    output = np.asarray(output, dtype=np.float32)
    target = np.asarray(target, dtype=np.float32)
    in_maps = []
    for i in range(N_CORES):
        sl = slice(i * ROWS_PER_CORE, (i + 1) * ROWS_PER_CORE)
        o4 = output[sl].reshape(N_TILES, P, FREE)
        t4 = target[sl].reshape(N_TILES, P, FREE)
        comb = np.stack([o4, t4], axis=1).reshape(N_TILES * 2 * P, FREE)
        in_maps.append({"combined": comb})
    return in_maps


def run_device(output, target, trace=False):
    """Returns (per-core partial moment arrays, BassKernelResults)."""
    in_maps = _shard_inputs(output, target)
    res = run_bass_kernel_spmd(_get_nc(), in_maps, list(range(N_CORES)), trace=trace)
    partials = [res.results[i]["partial"] for i in range(N_CORES)]
    return partials, res


def _reduce(partials):
    s1 = s2 = s3 = 0.0
    for p in partials:
        p64 = p.astype(np.float64)
        s1 += p64[:, 0:N_PIECES].sum()
        s2 += p64[:, N_PIECES : 2 * N_PIECES].sum()
        s3 += p64[:, 2 * N_PIECES :].sum()
    c = 4.0 * float(np.float32(1.0 / COLS))  # match reference's f32 sigma
    total = c * s1 - (c * c / 2.0) * s2 + (c * c * c / 6.0) * s3
    n = float(ROWS) * float(COLS)
    return np.array(total / n, dtype=np.float32)


def kernel(output, target):
    partials, _ = run_device(output, target)
    return _reduce(partials)
